# revision 4
# baseline (speedup 1.0000x reference)
"""CrissCrossAttention TRN2 kernel v2.

Shapes (hardcoded): x [16, 512, 96, 96] fp32, Wq/Wk [64, 512], Wv [512, 512],
biases, gamma [1]. 8 NeuronCores, data-parallel over batch (2 batches/core).

v2 structure (per batch):
  Stage 1 streams 6 x-tiles of [128, 4ci, 1536]:
    - fused q|k projection, single [128, 512] eviction per sub-block
    - val in 96-aligned chunks [96s, 512c] (one v-row per chunk)
    - phase-W scores (4 v's per PSUM group) + exp; denW via in-bank ones-matmul
    - phase-E output matmuls consume val chunks directly from SBUF
    - val chunks also stored to DRAM scratch vt2[t, v, c] (transposed layout)
  Stage 2 (after qk complete): loads vt2 tiles [96u, 8t, 512c], phase-H
    scores + exp + diag-mask, denH, phase-D output matmuls.
  Outputs: unnormalized bf16 partials oh/ow + fp32 denominators dh/dw.
  Host: out = x + gamma * (OH + OW) / (DH + DW).
"""

import numpy as np
import ml_dtypes

import concourse.bacc as bacc
import concourse.bass as bass
import concourse.tile as tile
from concourse import mybir
from concourse.bass_utils import run_bass_kernel_spmd

F32 = mybir.dt.float32
F16 = mybir.dt.float16
BF16 = mybir.dt.bfloat16
EXP = mybir.ActivationFunctionType.Exp
COPY = mybir.ActivationFunctionType.Copy
IDEN = mybir.ActivationFunctionType.Identity

B, C, V, T = 16, 512, 96, 96
C8 = 64
S = V * T            # 9216 spatial positions
NCORES = 8
BPC = B // NCORES    # batches per core
NCI = C // 128       # cin chunks
XB = 1536            # x-tile width (= 3*512 = 16*96)
NXT = S // XB        # 6 x-tiles per batch
SG = 4               # score tiles per PSUM group

_CACHE = {}


def _build(has_bv: bool, has_bqk: bool, repeat: int = 1):
    nc = bacc.Bacc("TRN2", target_bir_lowering=False, debug=False)

    xh = nc.dram_tensor("xh", [BPC, C, S], F16, kind="ExternalInput").ap()
    wqk = nc.dram_tensor("wqk", [NCI, 128, 128], F16, kind="ExternalInput").ap()
    wv4 = nc.dram_tensor("wv4", [NCI, 128, C], F16, kind="ExternalInput").ap()
    msk = nc.dram_tensor("msk", [V, SG * V], BF16, kind="ExternalInput").ap()
    if has_bqk:
        bqk = nc.dram_tensor("bqk", [128, 1], F32, kind="ExternalInput").ap()
    if has_bv:
        bv2 = nc.dram_tensor("bv2", [1, C], F32, kind="ExternalInput").ap()
    oh_d = nc.dram_tensor("oh", [BPC, S, C], BF16, kind="ExternalOutput").ap()
    ow_d = nc.dram_tensor("ow", [BPC, T, V, C], BF16, kind="ExternalOutput").ap()
    dh_d = nc.dram_tensor("dh", [BPC, V, T], F32, kind="ExternalOutput").ap()
    dw_d = nc.dram_tensor("dw", [BPC, T, V], F32, kind="ExternalOutput").ap()
    vt2 = nc.dram_tensor("vt2", [BPC, V, T, C], BF16).ap()

    with tile.TileContext(nc) as tc:
        with (
            tc.tile_pool(name="const", bufs=1) as cst,
            tc.tile_pool(name="xts", bufs=2) as xts,
            tc.tile_pool(name="qkp", bufs=2) as qkp,
            tc.tile_pool(name="stg", bufs=2) as stg,
            tc.tile_pool(name="den", bufs=2) as den,
            tc.tile_pool(name="ps", bufs=1, space="PSUM") as psp,
        ):
            wqk_sb = cst.tile([128, NCI, 128], F16)
            nc.sync.dma_start(out=wqk_sb, in_=wqk.rearrange("a p b -> p a b"))
            wv_sb = cst.tile([128, NCI, C], F16)
            nc.sync.dma_start(out=wv_sb, in_=wv4.rearrange("a p b -> p a b"))
            msk_sb = cst.tile([V, SG, V], BF16)
            nc.sync.dma_start(out=msk_sb,
                              in_=msk.rearrange("p (a b) -> p a b", a=SG))
            ones_sb = cst.tile([V, 1], BF16)
            nc.vector.memset(ones_sb, 1.0)
            if has_bqk:
                bqk_sb = cst.tile([128, 1], F32)
                nc.sync.dma_start(out=bqk_sb, in_=bqk)
            if has_bv:
                bv_sb = cst.tile([128, C], F32)
                nc.sync.dma_start(out=bv_sb, in_=bv2.to_broadcast([128, C]))

            ev = [0]  # eviction engine alternator

            def evict(dst, src):
                if ev[0] % 2 == 0:
                    nc.vector.tensor_copy(dst, src)
                else:
                    nc.scalar.activation(out=dst, in_=src, func=COPY)
                ev[0] += 1

            def stage1(b):
                """qk projection, val chunks, phase W+E. Yields (qk views)
                after allocating; then yields after each x-tile chunk."""
                x_b = xh[b].rearrange("(ci p) s -> p ci s", p=128)
                qk = qkp.tile([C8, 2, S], F16, tag="qk")
                q3 = qk[:, 0, :].rearrange("p (v t) -> p v t", t=T)
                k3 = qk[:, 1, :].rearrange("p (v t) -> p v t", t=T)
                swt = den.tile([T, V], F32, tag="sw")
                yield (q3, k3)
                for xti in range(NXT):
                    xt = xts.tile([128, NCI, XB], F16, tag="xt", bufs=3)
                    for sb in range(2):
                        nc.sync.dma_start(
                            out=xt[:, :, sb * 768 : (sb + 1) * 768],
                            in_=x_b[:, :, xti * XB + sb * 768
                                    : xti * XB + (sb + 1) * 768],
                        )
                    # q|k projection: 3 sub-blocks of 512
                    for sb in range(3):
                        pq = psp.tile([128, 512], F32, tag="mm", bufs=3)
                        for ci in range(NCI):
                            nc.tensor.matmul(
                                pq, wqk_sb[:, ci, :],
                                xt[:, ci, sb * 512 : (sb + 1) * 512],
                                start=(ci == 0), stop=(ci == NCI - 1),
                            )
                        js = slice(xti * XB + sb * 512,
                                   xti * XB + (sb + 1) * 512)
                        if has_bqk:
                            nc.scalar.activation(
                                out=qk[:, 0, js], in_=pq[0:C8, :], func=IDEN,
                                bias=bqk_sb[0:C8],
                            )
                            nc.scalar.activation(
                                out=qk[:, 1, js], in_=pq[C8:128, :], func=IDEN,
                                bias=bqk_sb[C8:128],
                            )
                        else:
                            evict(qk[:, 0, js], pq[0:C8, :])
                            evict(qk[:, 1, js], pq[C8:128, :])
                    # val chunks + phase W scores/dens + phase E outputs,
                    # 4 v's per group, 4 groups per x-tile
                    for g4 in range(4):
                        v0 = xti * 16 + g4 * 4
                        vv = stg.tile([V, SG, C], BF16, tag="vv", bufs=4)
                        for gg in range(SG):
                            pvt = psp.tile([128, 512], F32, tag="mm", bufs=3)
                            pv = pvt[0:V, :]
                            o0 = (g4 * 4 + gg) * V
                            for ci in range(NCI):
                                nc.tensor.matmul(
                                    pv, xt[:, ci, o0 : o0 + V],
                                    wv_sb[:, ci, :],
                                    start=(ci == 0), stop=(ci == NCI - 1),
                                )
                            if has_bv:
                                nc.vector.tensor_add(
                                    vv[:, gg, :], pv, bv_sb[0:V, :]
                                )
                            else:
                                evict(vv[:, gg, :], pv)
                        # W scores for 4 v's (+1 den column per v)
                        psw = psp.tile([V, SG, T + 1], F32, tag="ps", bufs=2)
                        for gg in range(SG):
                            nc.tensor.matmul(
                                psw[:, gg, 0:T], k3[:, v0 + gg, :],
                                q3[:, v0 + gg, :], start=True, stop=True,
                            )
                        pw = stg.tile([V, SG, T], BF16, tag="pw", bufs=3)
                        nc.scalar.activation(
                            out=pw, in_=psw[:, :, 0:T], func=EXP,
                        )
                        for gg in range(SG):
                            nc.tensor.matmul(
                                psw[:, gg, T : T + 1], pw[:, gg, :], ones_sb,
                                start=True, stop=True,
                            )
                        nc.vector.tensor_copy(
                            swt[:, v0 : v0 + SG], psw[:, :, T]
                        )
                        # phase E output matmuls (rhs = val chunk from SBUF);
                        # double-wide PSUM: one eviction per 2 matmuls
                        if g4 % 2 == 0:
                            osw = stg.tile([T, 8, C], BF16, tag="osw", bufs=3)
                        for gg in range(SG):
                            po = psp.tile([T, C], F32, tag="po", bufs=3)
                            nc.tensor.matmul(
                                po, pw[:, gg, :], vv[:, gg, :],
                                start=True, stop=True,
                            )
                            evict(osw[:, (g4 % 2) * SG + gg, :], po)
                        if g4 % 2 == 1:
                            nc.gpsimd.dma_start(
                                out=ow_d[b, :, v0 - 4 : v0 + 4, :],
                                in_=osw,
                            )
                        # val chunk -> transposed DRAM scratch vt2[t, v, c]
                        nc.sync.dma_start(
                            out=vt2[b][v0 : v0 + SG, :, :]
                            .rearrange("v t c -> t v c"),
                            in_=vv,
                        )
                    yield None
                nc.gpsimd.dma_start(out=dw_d[b], in_=swt)

            def stage2(b, q3, k3):
                """phase H scores/dens + phase D outputs; yields after each
                t0 chunk."""
                sh = den.tile([V, T], F32, tag="sh")
                for t0 in range(0, T, 8):
                    vd = stg.tile([V, 8, C], BF16, tag="vd", bufs=2)
                    nc.sync.dma_start(
                        out=vd, in_=vt2[b, :, t0 : t0 + 8, :],
                    )
                    osh = stg.tile([V, 8, C], BF16, tag="osh", bufs=3)
                    for h in range(2):
                        psh = psp.tile([V, SG, V + 1], F32, tag="ps", bufs=2)
                        for gg in range(SG):
                            t = t0 + h * SG + gg
                            nc.tensor.matmul(
                                psh[:, gg, 0:V], k3[:, :, t], q3[:, :, t],
                                start=True, stop=True,
                            )
                        ph = stg.tile([V, SG, V], BF16, tag="pw", bufs=3)
                        nc.scalar.activation(
                            out=ph, in_=psh[:, :, 0:V], func=EXP,
                        )
                        nc.vector.tensor_mul(ph, ph, msk_sb)
                        for gg in range(SG):
                            nc.tensor.matmul(
                                psh[:, gg, V : V + 1], ph[:, gg, :], ones_sb,
                                start=True, stop=True,
                            )
                        nc.vector.tensor_copy(
                            sh[:, t0 + h * SG : t0 + (h + 1) * SG],
                            psh[:, :, V],
                        )
                        for gg in range(SG):
                            po = psp.tile([V, C], F32, tag="po", bufs=3)
                            nc.tensor.matmul(
                                po, ph[:, gg, :], vd[:, h * SG + gg, :],
                                start=True, stop=True,
                            )
                            evict(osh[:, h * SG + gg, :], po)
                    nc.gpsimd.dma_start(
                        out=oh_d[b].rearrange("(v t) c -> v t c", t=T)
                        [:, t0 : t0 + 8, :],
                        in_=osh,
                    )
                    yield None
                nc.gpsimd.dma_start(out=dh_d[b], in_=sh)

            # Software pipeline: interleave batch b's stage 2 with batch
            # b+1's stage 1 (2 t0-chunks per x-tile chunk) so engines can
            # fill each other's stalls.
            prev_s2 = None
            for b in [bb for _ in range(repeat) for bb in range(BPC)]:
                s1 = stage1(b)
                q3, k3 = next(s1)
                for _ in range(NXT):
                    next(s1, None)
                    if prev_s2 is not None:
                        next(prev_s2, None)
                        next(prev_s2, None)
                next(s1, None)  # run stage-1 epilogue (dw store)
                if prev_s2 is not None:
                    for _ in prev_s2:
                        pass
                prev_s2 = stage2(b, q3, k3)
            for _ in prev_s2:
                pass

    nc.compile()
    return nc


def _prep_inputs(x, Wq, bq, Wk, bk, Wv, bv, gamma):
    x16 = np.ascontiguousarray(x.reshape(B, C, S)).astype(np.float16)
    wqk = np.concatenate([Wq, Wk], axis=0).T.reshape(NCI, 128, 128)
    wqk = np.ascontiguousarray(wqk).astype(np.float16)
    wv4 = np.ascontiguousarray(Wv.T.reshape(NCI, 128, C)).astype(np.float16)
    mask = np.tile((1.0 - np.eye(V)), (1, SG)).astype(ml_dtypes.bfloat16)
    shared = {"wqk": wqk, "wv4": wv4, "msk": mask}
    if np.any(bq) or np.any(bk):
        shared["bqk"] = np.concatenate([bq, bk]).reshape(128, 1).astype(np.float32)
    if np.any(bv):
        shared["bv2"] = bv.reshape(1, C).astype(np.float32)
    in_maps = [
        {"xh": x16[i * BPC : (i + 1) * BPC], **shared} for i in range(NCORES)
    ]
    return in_maps


def _run(inputs, trace=False, trace_kwargs=None):
    has_bv = bool(np.any(inputs["bv"]))
    has_bqk = bool(np.any(inputs["bq"])) or bool(np.any(inputs["bk"]))
    key = ("nc", has_bv, has_bqk)
    if key not in _CACHE:
        _CACHE[key] = _build(has_bv, has_bqk)
    nc = _CACHE[key]
    in_maps = _prep_inputs(**inputs)
    res = run_bass_kernel_spmd(
        nc, in_maps, list(range(NCORES)), trace=trace,
        **(trace_kwargs or {}),
    )
    oh = np.concatenate([res.results[i]["oh"] for i in range(NCORES)], axis=0)
    ow4 = np.concatenate([res.results[i]["ow"] for i in range(NCORES)], axis=0)
    ow = np.ascontiguousarray(ow4.transpose(0, 2, 1, 3)).reshape(B, S, C)
    dh = np.concatenate([res.results[i]["dh"] for i in range(NCORES)], axis=0)
    dw = np.concatenate([res.results[i]["dw"] for i in range(NCORES)], axis=0)
    part = oh.astype(np.float32) + ow.astype(np.float32)     # [B, S, C]
    dsum = dh + dw.transpose(0, 2, 1)                        # [B, V, T]
    part /= dsum.reshape(B, S, 1)
    gamma = float(inputs["gamma"].reshape(-1)[0])
    out = inputs["x"].reshape(B, C, S) + gamma * part.transpose(0, 2, 1)
    return out.reshape(B, C, V, T).astype(np.float32), res


def kernel(**inputs):
    out, _ = _run(inputs)
    return out
